# revision 13
# baseline (speedup 1.0000x reference)
"""GCN (2-layer, symmetric-norm message passing) on 8 Trainium2 NeuronCores.

Contract: kernel(**inputs) takes the FULL inputs (x [50000,4,300] f32,
edge_index [2,250000] i32, W1/b1/W2/b2) and returns the FULL output
[50000,300] f32.

Strategy (per sharding hint): shard destination nodes across the 8 cores
(6250 each), replicate the small weights, partition edges by destination so
scatter-adds are core-local, and AllGather the pre-scaled source features
between layers.  The scatter-add is computed on the PE array as 0/1-indicator
matmuls over 128-edge chunks (edges sorted by destination on the host); the
per-edge feature gather uses the DMAGatherAnt custom instruction.

v3.1:
  - x cast to bf16 on the host (device compute was already bf16).
  - gathers via nc.gpsimd.dma_gather (int16 indices, 768B padded rows),
    batched over several destination blocks per instruction with at most
    1024 indices per op (the SWDGE descriptor ring can't take 2048).
  - per-(block,slice) chunk counts are exact (no uniform-CPB padding).
  - sources are split into 3 row-slices, each AllGathered into its own
    Shared tensor as soon as its producing blocks finish, overlapping the
    collectives with compute.  The prop runs in two passes: pass 1
    accumulates slices 0+1 into an SBUF accumulator (with the self term),
    pass 2 adds slice 2, hiding the tail collective.
"""

import numpy as np

import concourse.bacc as bacc
import concourse.bass as bass
import concourse.tile as tile
from concourse import bass_utils, mybir
from concourse.masks import make_identity

F32 = mybir.dt.float32
BF16 = mybir.dt.bfloat16
I16 = mybir.dt.int16
P = 128

N_CORES = 8
N_SLICES = 3
MAXCH = 8  # max 128-index chunks per dma_gather op (1024 idx, ring-safe)
CE = 384   # padded feature row (384 bf16 = 768 B, must be %256 B)


def _cdiv(a, b):
    return (a + b - 1) // b


# ---------------------------------------------------------------- host prep


def _slice_plan(NPC, NBLK, n_slices):
    per = _cdiv(NBLK, n_slices)
    R = [0]
    for s in range(n_slices):
        b1 = min((s + 1) * per, NBLK)
        R.append(min(b1 * P, NPC))
    SL = [R[s + 1] - R[s] for s in range(n_slices)]
    return R, SL


def _wrap_idx16(flat):
    L = len(flat)
    assert L % 16 == 0
    w = np.zeros((16, L // 16), np.int16)
    w[np.arange(L) % 16, np.arange(L) // 16] = flat.astype(np.int16)
    return np.tile(w, (8, 1))


def prep_inputs(x, edge_index, W1, b1, W2, b2, n_cores=N_CORES):
    import ml_dtypes

    N, T, C = x.shape
    assert N % n_cores == 0
    NPC = N // n_cores
    NBLK = _cdiv(NPC, P)
    R, SL = _slice_plan(NPC, NBLK, N_SLICES)
    assert max(SL) * n_cores < 2**15, "dma_gather indices must fit int16"

    row = np.asarray(edge_index[0], dtype=np.int64)
    col = np.asarray(edge_index[1], dtype=np.int64)

    deg = (np.bincount(row, minlength=N) + 1).astype(np.float32)
    dis = (deg.astype(np.float32) ** -0.5).astype(np.float32)

    r_of = row // NPC
    i_of = row % NPC
    s_of = np.zeros_like(i_of)
    for s in range(1, N_SLICES):
        s_of += i_of >= R[s]
    SLa = np.asarray(SL, dtype=np.int64)
    Ra = np.asarray(R[:-1], dtype=np.int64)
    idx_in_slice = r_of * SLa[s_of] + (i_of - Ra[s_of])

    core_of = col // NPC

    # per (core, slice): edges sorted by dest; per-block counts
    percore = []
    # nch[s][b]: chunks for (slice, block) — shared across cores so the
    # device program is SPMD-identical
    cnt_all = np.zeros((n_cores, N_SLICES, NBLK), np.int64)
    for c in range(n_cores):
        m = core_of == c
        entries = []
        for s in range(N_SLICES):
            ms = m & (s_of == s)
            idv = idx_in_slice[ms]
            d = col[ms] - c * NPC
            order = np.argsort(d, kind="stable")
            idv = idv[order]
            d = d[order]
            cnt = np.bincount(d // P, minlength=NBLK)
            cnt_all[c, s] = cnt
            entries.append((idv, d, cnt))
        percore.append(entries)
    # chunks per (slice, block): max over cores so tables align SPMD
    nch = np.maximum(1, _cdiv(cnt_all.max(axis=0), P))  # [N_SLICES, NBLK]
    choff = np.zeros((N_SLICES, NBLK + 1), np.int64)
    choff[:, 1:] = np.cumsum(nch, axis=1)

    # gather groups: pass1 shared over slices 0..N_SLICES-2, pass2 last slice
    def make_groups(slices):
        grp = []
        b0 = 0
        while b0 < NBLK:
            b1 = b0 + 1
            while b1 < NBLK and all(
                choff[s, b1 + 1] - choff[s, b0] <= MAXCH for s in slices
            ):
                b1 += 1
            grp.append((b0, b1))
            b0 = b1
        return grp
    groups1 = make_groups(range(N_SLICES - 1))
    groups2 = make_groups([N_SLICES - 1])

    CC = [(c0, min(P, C - c0)) for c0 in range(0, C, P)]
    KC = len(CC)
    w1c = np.zeros((KC, P, C), ml_dtypes.bfloat16)
    w2c = np.zeros((KC, P, C), ml_dtypes.bfloat16)
    for k, (c0, cs) in enumerate(CC):
        w1c[k, :cs, :] = (W1.T[c0 : c0 + cs, :] / np.float32(T)).astype(np.float32)
        w2c[k, :cs, :] = W2.T[c0 : c0 + cs, :].astype(np.float32)
    b1t = np.broadcast_to(np.asarray(b1, np.float32), (P, C)).copy()
    b2t = np.broadcast_to(np.asarray(b2, np.float32), (P, C)).copy()
    iota = np.broadcast_to(np.arange(P, dtype=np.float32), (P, P)).astype(
        ml_dtypes.bfloat16
    )

    in_maps = []
    for c in range(n_cores):
        imap = {"w1c": w1c, "w2c": w2c, "b1t": b1t, "b2t": b2t, "iot": iota}
        for s in range(N_SLICES):
            idv, d, cnt = percore[c][s]
            starts = np.concatenate([[0], np.cumsum(cnt)])
            slots = int(choff[s, -1]) * P
            ids_flat = np.zeros(slots, np.int64)
            dl_flat = np.full(slots, -1.0, np.float32)
            for blk in range(NBLK):
                s0, e0 = int(starts[blk]), int(starts[blk + 1])
                n = e0 - s0
                o = int(choff[s, blk]) * P
                ids_flat[o : o + n] = idv[s0:e0]
                dl_flat[o : o + n] = (d[s0:e0] - blk * P).astype(np.float32)
            imap[f"idx16_{s}"] = _wrap_idx16(ids_flat)
            imap[f"dlt_{s}"] = (
                dl_flat.reshape(-1, P).T.astype(ml_dtypes.bfloat16).copy()
            )

        dis_c = dis[c * NPC : (c + 1) * NPC]
        dist = np.zeros((P, NBLK), np.float32)
        flat = np.zeros(NBLK * P, np.float32)
        flat[:NPC] = dis_c
        dist[:, :] = flat.reshape(NBLK, P).T
        imap["dist"] = dist
        imap["xs"] = np.ascontiguousarray(x[c * NPC : (c + 1) * NPC]).astype(
            ml_dtypes.bfloat16
        )
        in_maps.append(imap)

    meta = dict(
        N=N, T=T, C=C, NPC=NPC, NBLK=NBLK, CC=CC, n_cores=n_cores,
        R=R, SL=SL, nch=nch, choff=choff, groups1=groups1, groups2=groups2,
    )
    return in_maps, meta


# ------------------------------------------------------------- device build


def build_nc(meta):
    N = meta["N"]
    T = meta["T"]
    C = meta["C"]
    NPC = meta["NPC"]
    NBLK = meta["NBLK"]
    CC = meta["CC"]
    KC = len(CC)
    n_cores = meta["n_cores"]
    R = meta["R"]
    SL = meta["SL"]
    nch = meta["nch"]
    choff = meta["choff"]
    groups1 = meta["groups1"]
    groups2 = meta["groups2"]
    rg = [list(range(n_cores))]
    sLast = N_SLICES - 1

    nc = bacc.Bacc(
        "TRN2", target_bir_lowering=False, debug=False, num_devices=n_cores,
        num_swdge_queues=4,
    )

    xs = nc.dram_tensor("xs", [NPC, T, C], BF16, kind="ExternalInput")
    w1c = nc.dram_tensor("w1c", [KC, P, C], BF16, kind="ExternalInput")
    w2c = nc.dram_tensor("w2c", [KC, P, C], BF16, kind="ExternalInput")
    b1t = nc.dram_tensor("b1t", [P, C], F32, kind="ExternalInput")
    b2t = nc.dram_tensor("b2t", [P, C], F32, kind="ExternalInput")
    iot = nc.dram_tensor("iot", [P, P], BF16, kind="ExternalInput")
    dist = nc.dram_tensor("dist", [P, NBLK], F32, kind="ExternalInput")
    idx16 = [
        nc.dram_tensor(
            f"idx16_{s}", [P, int(choff[s, -1]) * 8], I16, kind="ExternalInput"
        )
        for s in range(N_SLICES)
    ]
    dlt = [
        nc.dram_tensor(
            f"dlt_{s}", [P, int(choff[s, -1])], BF16, kind="ExternalInput"
        )
        for s in range(N_SLICES)
    ]
    out_ext = nc.dram_tensor("out", [NPC, C], F32, kind="ExternalOutput")

    ACT = mybir.ActivationFunctionType
    slice_after = {_cdiv(R[s + 1], P) - 1: s for s in range(N_SLICES)}

    with tile.TileContext(nc) as tc:
        with (
            tc.tile_pool(name="dramp", bufs=1, space="DRAM") as dramp,
            tc.tile_pool(name="singles", bufs=1) as singles,
            tc.tile_pool(name="work", bufs=3) as wp,
            tc.tile_pool(name="msgs", bufs=3) as mp,
            tc.tile_pool(name="psA", bufs=2, space="PSUM") as psA,
            tc.tile_pool(name="psT", bufs=2, space="PSUM") as psT,
            tc.tile_pool(name="psB", bufs=2, space="PSUM") as psB,
            tc.tile_pool(name="psC", bufs=2, space="PSUM") as psC,
        ):
            agin1 = [
                dramp.tile([SL[s], CE], BF16, name=f"agin1_{s}")
                for s in range(N_SLICES)
            ]
            hp1f = [
                dramp.tile(
                    [n_cores * SL[s], CE], BF16, addr_space="Shared",
                    name=f"hp1f_{s}",
                )
                for s in range(N_SLICES)
            ]
            agin2 = [
                dramp.tile([SL[s], CE], BF16, name=f"agin2_{s}")
                for s in range(N_SLICES)
            ]
            hp2f = [
                dramp.tile(
                    [n_cores * SL[s], CE], BF16, addr_space="Shared",
                    name=f"hp2f_{s}",
                )
                for s in range(N_SLICES)
            ]

            ident = singles.tile([P, P], BF16, name="ident")
            make_identity(nc, ident[:])
            w1sb = singles.tile([P, KC, C], BF16, name="w1sb")
            w2sb = singles.tile([P, KC, C], BF16, name="w2sb")
            for k in range(KC):
                nc.sync.dma_start(out=w1sb[:, k, :], in_=w1c[k])
                nc.sync.dma_start(out=w2sb[:, k, :], in_=w2c[k])
            b1sb = singles.tile([P, C], F32, name="b1sb")
            nc.sync.dma_start(out=b1sb[:], in_=b1t[:])
            b2sb = singles.tile([P, C], F32, name="b2sb")
            nc.sync.dma_start(out=b2sb[:], in_=b2t[:])
            iosb = singles.tile([P, P], BF16, name="iosb")
            nc.sync.dma_start(out=iosb[:], in_=iot[:])
            dissb = singles.tile([P, NBLK], F32, name="dissb")
            nc.sync.dma_start(out=dissb[:], in_=dist[:])
            idxsb = []
            dlsb = []
            for s in range(N_SLICES):
                t = singles.tile(
                    [P, int(choff[s, -1]) * 8], I16, name=f"idxsb{s}"
                )
                nc.sync.dma_start(out=t[:], in_=idx16[s][:])
                idxsb.append(t)
                t2 = singles.tile([P, int(choff[s, -1])], BF16, name=f"dlsb{s}")
                nc.sync.dma_start(out=t2[:], in_=dlt[s][:])
                dlsb.append(t2)

            hps1 = singles.tile([P, NBLK, C], BF16, name="hps1")
            hps2 = singles.tile([P, NBLK, C], BF16, name="hps2")
            hacc = singles.tile([P, NBLK, C], F32, name="hacc")
            if NPC % P != 0:
                nc.vector.memset(hps1[:], 0.0)
                nc.vector.memset(hps2[:], 0.0)

            def ag_slice(agin, hpf, s):
                nc.gpsimd.collective_compute(
                    "AllGather",
                    mybir.AluOpType.bypass,
                    replica_groups=rg,
                    ins=[agin[s][:].opt()],
                    outs=[hpf[s][:].opt()],
                )

            def agin_write(agin, b, Pb, src):
                s = 0
                while R[s + 1] <= b * P:
                    s += 1
                o = b * P - R[s]
                nc.sync.dma_start(out=agin[s][o : o + Pb], in_=src)

            # ---------------- stage A: h = mean_t(x) @ W1.T + b1, prescale
            for b in range(NBLK):
                Pb = min(P, NPC - b * P)
                dcol = dissb[:Pb, b : b + 1]
                xt = wp.tile([P, T, C], BF16, tag="xt")
                nc.sync.dma_start(out=xt[:Pb], in_=xs[b * P : b * P + Pb])
                s0 = wp.tile([P, C], BF16, tag="s0")
                s1 = wp.tile([P, C], BF16, tag="s1")
                xm = wp.tile([P, C], BF16, tag="xm")
                nc.vector.tensor_add(out=s0[:Pb], in0=xt[:Pb, 0], in1=xt[:Pb, 1])
                nc.vector.tensor_add(out=s1[:Pb], in0=xt[:Pb, 2], in1=xt[:Pb, 3])
                nc.vector.tensor_add(out=xm[:Pb], in0=s0[:Pb], in1=s1[:Pb])
                hpp = psA.tile([P, C], F32, tag="hpp")
                for k, (c0, cs) in enumerate(CC):
                    ptr = psT.tile([P, P], BF16, tag="ptr")
                    nc.tensor.transpose(
                        out=ptr[:cs, :Pb],
                        in_=xm[:Pb, c0 : c0 + cs],
                        identity=ident[:Pb, :Pb],
                    )
                    xT = wp.tile([P, P], BF16, tag="xT")
                    nc.scalar.copy(out=xT[:cs, :Pb], in_=ptr[:cs, :Pb])
                    nc.tensor.matmul(
                        out=hpp[:Pb],
                        lhsT=xT[:cs, :Pb],
                        rhs=w1sb[:cs, k, :],
                        start=(k == 0),
                        stop=(k == KC - 1),
                    )
                th = wp.tile([P, C], F32, tag="th")
                nc.vector.tensor_add(out=th[:Pb], in0=hpp[:Pb], in1=b1sb[:Pb])
                hp_t = wp.tile([P, CE], BF16, tag="hp")
                nc.vector.memset(hp_t[:Pb, C:], 0.0)
                nc.scalar.activation(
                    out=hp_t[:Pb, :C], in_=th[:Pb], func=ACT.Copy, scale=dcol
                )
                agin_write(agin1, b, Pb, hp_t[:Pb])
                nc.scalar.activation(
                    out=hps1[:Pb, b, :], in_=hp_t[:Pb, :C], func=ACT.Copy, scale=dcol
                )
                if b in slice_after:
                    ag_slice(agin1, hp1f, slice_after[b])

            # ------------- gather slice s chunks for blocks [g0, g1)
            qn = [0]

            def gather(hpf, s, g0, g1):
                c0, c1 = int(choff[s, g0]), int(choff[s, g1])
                ncols = c1 - c0
                msg = mp.tile([P, MAXCH, CE], BF16, tag="msg")
                nidx = ncols * P
                nc.gpsimd.dma_gather(
                    out_ap=msg[:, :ncols, :],
                    in_ap=hpf[s][:],
                    idxs_ap=idxsb[s][:, c0 * 8 : c1 * 8],
                    num_idxs=nidx,
                    num_idxs_reg=nidx,
                    elem_size=CE,
                    queue_num=qn[0],
                )
                qn[0] = (qn[0] + 1) % 4
                return msg

            # all indicators for blocks [g0, g1) of slice s in one DVE op
            def indicators(s, g0, g1):
                c0, c1 = int(choff[s, g0]), int(choff[s, g1])
                ncols = c1 - c0
                indall = mp.tile([P, MAXCH, P], BF16, tag="indall")
                nc.vector.tensor_tensor(
                    out=indall[:, :ncols, :],
                    in0=iosb[:].unsqueeze(1).to_broadcast([P, ncols, P]),
                    in1=dlsb[s][:, c0:c1].unsqueeze(2).to_broadcast(
                        [P, ncols, P]
                    ),
                    op=mybir.AluOpType.is_equal,
                )
                return indall

            def prop_pass1(hpf, hps):
                for g0, g1 in groups1:
                    msgs = [gather(hpf, s, g0, g1) for s in range(N_SLICES - 1)]
                    inds = [
                        indicators(s, g0, g1) for s in range(N_SLICES - 1)
                    ]
                    for b in range(g0, g1):
                        pp = psB.tile([P, C], F32, tag="pp")
                        first = True
                        for s in range(N_SLICES - 1):
                            mcol0 = int(choff[s, b]) - int(choff[s, g0])
                            for ch in range(int(nch[s, b])):
                                last = (
                                    s == N_SLICES - 2
                                    and ch == int(nch[s, b]) - 1
                                )
                                nc.tensor.matmul(
                                    out=pp[:],
                                    lhsT=inds[s][:, mcol0 + ch, :],
                                    rhs=msgs[s][:, mcol0 + ch, :C],
                                    start=first,
                                    stop=last,
                                )
                                first = False
                        nc.vector.scalar_tensor_tensor(
                            out=hacc[:, b, :],
                            in0=pp[:],
                            scalar=dissb[:, b : b + 1],
                            in1=hps[:, b, :],
                            op0=mybir.AluOpType.mult,
                            op1=mybir.AluOpType.add,
                        )

            def prop_pass2_block(msg, indall, g0, b):
                pp = psB.tile([P, C], F32, tag="pp")
                mcol0 = int(choff[sLast, b]) - int(choff[sLast, g0])
                nb = int(nch[sLast, b])
                for ch in range(nb):
                    nc.tensor.matmul(
                        out=pp[:],
                        lhsT=indall[:, mcol0 + ch, :],
                        rhs=msg[:, mcol0 + ch, :C],
                        start=(ch == 0),
                        stop=(ch == nb - 1),
                    )
                t1 = wp.tile([P, C], F32, tag="t1")
                nc.vector.scalar_tensor_tensor(
                    out=t1[:],
                    in0=pp[:],
                    scalar=dissb[:, b : b + 1],
                    in1=hacc[:, b, :],
                    op0=mybir.AluOpType.mult,
                    op1=mybir.AluOpType.add,
                )
                return t1

            # ---------------- layer 1 prop + layer 2 linear
            prop_pass1(hp1f, hps1)
            for g0, g1 in groups2:
                msg = gather(hp1f, sLast, g0, g1)
                indall = indicators(sLast, g0, g1)
                for b in range(g0, g1):
                    Pb = min(P, NPC - b * P)
                    t1 = prop_pass2_block(msg, indall, g0, b)
                    h1 = wp.tile([P, C], BF16, tag="h1")
                    nc.vector.scalar_tensor_tensor(
                        out=h1[:],
                        in0=t1[:],
                        scalar=0.01,
                        in1=t1[:],
                        op0=mybir.AluOpType.mult,
                        op1=mybir.AluOpType.max,
                    )
                    h2p = psC.tile([P, C], F32, tag="h2p")
                    for k, (c0, cs) in enumerate(CC):
                        ptr2 = psT.tile([P, P], BF16, tag="ptr")
                        nc.tensor.transpose(
                            out=ptr2[:cs, :],
                            in_=h1[:, c0 : c0 + cs],
                            identity=ident[:],
                        )
                        hT = wp.tile([P, P], BF16, tag="hT")
                        nc.scalar.copy(out=hT[:cs, :], in_=ptr2[:cs, :])
                        nc.tensor.matmul(
                            out=h2p[:],
                            lhsT=hT[:cs, :],
                            rhs=w2sb[:cs, k, :],
                            start=(k == 0),
                            stop=(k == KC - 1),
                        )
                    t2 = wp.tile([P, C], F32, tag="t2")
                    nc.vector.tensor_add(out=t2[:], in0=h2p[:], in1=b2sb[:])
                    hp2_t = wp.tile([P, CE], BF16, tag="hp2")
                    nc.vector.memset(hp2_t[:Pb, C:], 0.0)
                    nc.scalar.activation(
                        out=hp2_t[:Pb, :C],
                        in_=t2[:Pb],
                        func=ACT.Copy,
                        scale=dissb[:Pb, b : b + 1],
                    )
                    agin_write(agin2, b, Pb, hp2_t[:Pb])
                    nc.scalar.activation(
                        out=hps2[:Pb, b, :],
                        in_=hp2_t[:Pb, :C],
                        func=ACT.Copy,
                        scale=dissb[:Pb, b : b + 1],
                    )
                    if b in slice_after:
                        ag_slice(agin2, hp2f, slice_after[b])

            # ---------------- layer 2 prop -> output
            prop_pass1(hp2f, hps2)
            for g0, g1 in groups2:
                msg = gather(hp2f, sLast, g0, g1)
                indall = indicators(sLast, g0, g1)
                for b in range(g0, g1):
                    Pb = min(P, NPC - b * P)
                    ot = prop_pass2_block(msg, indall, g0, b)
                    nc.sync.dma_start(out=out_ext[b * P : b * P + Pb], in_=ot[:Pb])

    nc.compile()
    return nc


# ------------------------------------------------------------------ runner

_CACHE = {}


def run(x, edge_index, W1, b1, W2, b2, n_cores=N_CORES, trace=False):
    in_maps, meta = prep_inputs(x, edge_index, W1, b1, W2, b2, n_cores)
    key = (
        meta["N"], meta["T"], meta["C"], n_cores,
        tuple(map(tuple, meta["nch"])),
    )
    if key not in _CACHE:
        _CACHE[key] = build_nc(meta)
    nc = _CACHE[key]
    res = bass_utils.run_bass_kernel_spmd(
        nc, in_maps, core_ids=list(range(n_cores)), trace=trace
    )
    outs = [np.asarray(res.results[c]["out"]) for c in range(n_cores)]
    full = np.concatenate(outs, axis=0).astype(np.float32)
    return full, res


def kernel(x, edge_index, W1, b1, W2, b2):
    x = np.asarray(x)
    edge_index = np.asarray(edge_index)
    full, _ = run(
        np.asarray(x, np.float32),
        edge_index,
        np.asarray(W1, np.float32),
        np.asarray(b1, np.float32),
        np.asarray(W2, np.float32),
        np.asarray(b2, np.float32),
    )
    return full


# revision 17
# speedup vs baseline: 1.1489x; 1.1489x over previous
"""GCN (2-layer, symmetric-norm message passing) on 8 Trainium2 NeuronCores.

Contract: kernel(**inputs) takes the FULL inputs (x [50000,4,300] f32,
edge_index [2,250000] i32, W1/b1/W2/b2) and returns the FULL output
[50000,300] f32.

Strategy (per sharding hint): shard destination nodes across the 8 cores
(6250 each), replicate the small weights, partition edges by destination so
scatter-adds are core-local, and AllGather the pre-scaled source features
between layers.  The scatter-add is computed on the PE array as 0/1-indicator
matmuls over 128-edge chunks (edges sorted by destination on the host); the
per-edge feature gather uses the DMAGatherAnt custom instruction.

v3.1:
  - x cast to bf16 on the host (device compute was already bf16).
  - gathers via nc.gpsimd.dma_gather (int16 indices, 768B padded rows),
    batched over several destination blocks per instruction with at most
    1024 indices per op (the SWDGE descriptor ring can't take 2048).
  - per-(block,slice) chunk counts are exact (no uniform-CPB padding).
  - sources are split into 3 row-slices, each AllGathered into its own
    Shared tensor as soon as its producing blocks finish, overlapping the
    collectives with compute.  The prop runs in two passes: pass 1
    accumulates slices 0+1 into an SBUF accumulator (with the self term),
    pass 2 adds slice 2, hiding the tail collective.
"""

import numpy as np

import concourse.bacc as bacc
import concourse.bass as bass
import concourse.tile as tile
from concourse import bass_utils, mybir
from concourse.masks import make_identity

F32 = mybir.dt.float32
BF16 = mybir.dt.bfloat16
I16 = mybir.dt.int16
P = 128

N_CORES = 8
N_SLICES = 3
MAXCH = 8  # max 128-index chunks per dma_gather op (1024 idx, ring-safe)
CE = 384   # padded feature row (384 bf16 = 768 B, must be %256 B)


def _cdiv(a, b):
    return (a + b - 1) // b


# ---------------------------------------------------------------- host prep


def _slice_plan(NPC, NBLK, n_slices):
    per = _cdiv(NBLK, n_slices)
    R = [0]
    for s in range(n_slices):
        b1 = min((s + 1) * per, NBLK)
        R.append(min(b1 * P, NPC))
    SL = [R[s + 1] - R[s] for s in range(n_slices)]
    return R, SL


def _wrap_idx16(flat):
    L = len(flat)
    assert L % 16 == 0
    w = np.zeros((16, L // 16), np.int16)
    w[np.arange(L) % 16, np.arange(L) // 16] = flat.astype(np.int16)
    return np.tile(w, (8, 1))


def prep_inputs(x, edge_index, W1, b1, W2, b2, n_cores=N_CORES):
    import ml_dtypes

    N, T, C = x.shape
    assert N % n_cores == 0
    NPC = N // n_cores
    NBLK = _cdiv(NPC, P)
    R, SL = _slice_plan(NPC, NBLK, N_SLICES)
    assert max(SL) * n_cores < 2**15, "dma_gather indices must fit int16"

    row = np.asarray(edge_index[0], dtype=np.int64)
    col = np.asarray(edge_index[1], dtype=np.int64)

    deg = (np.bincount(row, minlength=N) + 1).astype(np.float32)
    dis = (deg.astype(np.float32) ** -0.5).astype(np.float32)

    r_of = row // NPC
    i_of = row % NPC
    s_of = np.zeros_like(i_of)
    for s in range(1, N_SLICES):
        s_of += i_of >= R[s]
    SLa = np.asarray(SL, dtype=np.int64)
    Ra = np.asarray(R[:-1], dtype=np.int64)
    idx_in_slice = r_of * SLa[s_of] + (i_of - Ra[s_of])

    core_of = col // NPC

    # per (core, slice): edges sorted by dest; per-block counts
    percore = []
    # nch[s][b]: chunks for (slice, block) — shared across cores so the
    # device program is SPMD-identical
    cnt_all = np.zeros((n_cores, N_SLICES, NBLK), np.int64)
    for c in range(n_cores):
        m = core_of == c
        entries = []
        for s in range(N_SLICES):
            ms = m & (s_of == s)
            idv = idx_in_slice[ms]
            d = col[ms] - c * NPC
            order = np.argsort(d, kind="stable")
            idv = idv[order]
            d = d[order]
            cnt = np.bincount(d // P, minlength=NBLK)
            cnt_all[c, s] = cnt
            entries.append((idv, d, cnt))
        percore.append(entries)
    # chunks per (slice, block): max over cores so tables align SPMD
    nch = np.maximum(1, _cdiv(cnt_all.max(axis=0), P))  # [N_SLICES, NBLK]
    choff = np.zeros((N_SLICES, NBLK + 1), np.int64)
    choff[:, 1:] = np.cumsum(nch, axis=1)

    # gather groups: pass1 shared over slices 0..N_SLICES-2, pass2 last slice
    def make_groups(slices):
        grp = []
        b0 = 0
        while b0 < NBLK:
            b1 = b0 + 1
            while b1 < NBLK and all(
                choff[s, b1 + 1] - choff[s, b0] <= MAXCH for s in slices
            ):
                b1 += 1
            grp.append((b0, b1))
            b0 = b1
        return grp
    groups1 = make_groups(range(N_SLICES - 1))
    groups2 = make_groups([N_SLICES - 1])

    CK = 100  # contraction chunk (C = 3*CK)
    CC = [(c0, CK) for c0 in range(0, C, CK)]
    KC = len(CC)
    w1c = np.zeros((KC, P, C), ml_dtypes.bfloat16)
    w2c = np.zeros((KC, P, C), ml_dtypes.bfloat16)
    for k, (c0, cs) in enumerate(CC):
        w1c[k, :cs, :] = (W1.T[c0 : c0 + cs, :] / np.float32(T)).astype(np.float32)
        w2c[k, :cs, :] = W2.T[c0 : c0 + cs, :].astype(np.float32)
    b1t = np.broadcast_to(np.asarray(b1, np.float32), (P, C)).copy()
    b2t = np.broadcast_to(np.asarray(b2, np.float32), (P, C)).copy()
    iota = np.broadcast_to(np.arange(P, dtype=np.float32), (P, P)).astype(
        ml_dtypes.bfloat16
    )

    in_maps = []
    for c in range(n_cores):
        imap = {"w1c": w1c, "w2c": w2c, "b1t": b1t, "b2t": b2t, "iot": iota}
        for s in range(N_SLICES):
            idv, d, cnt = percore[c][s]
            starts = np.concatenate([[0], np.cumsum(cnt)])
            slots = int(choff[s, -1]) * P
            ids_flat = np.zeros(slots, np.int64)
            dl_flat = np.full(slots, -1.0, np.float32)
            for blk in range(NBLK):
                s0, e0 = int(starts[blk]), int(starts[blk + 1])
                n = e0 - s0
                o = int(choff[s, blk]) * P
                ids_flat[o : o + n] = idv[s0:e0]
                dl_flat[o : o + n] = (d[s0:e0] - blk * P).astype(np.float32)
            imap[f"idx16_{s}"] = _wrap_idx16(ids_flat)
            imap[f"dlt_{s}"] = (
                dl_flat.reshape(-1, P).T.astype(ml_dtypes.bfloat16).copy()
            )

        dis_c = dis[c * NPC : (c + 1) * NPC]
        dist = np.zeros((P, NBLK), np.float32)
        flat = np.zeros(NBLK * P, np.float32)
        flat[:NPC] = dis_c
        dist[:, :] = flat.reshape(NBLK, P).T
        imap["dist"] = dist
        xc = np.asarray(x[c * NPC : (c + 1) * NPC])  # [NPC, T, C]
        xsT = (
            xc.transpose(2, 1, 0)
            .reshape(3, CK, T, NPC)
            .transpose(1, 2, 0, 3)
            .reshape(CK, 3 * T, NPC)
        )
        imap["xsT"] = np.ascontiguousarray(xsT).astype(ml_dtypes.bfloat16)
        in_maps.append(imap)

    meta = dict(
        N=N, T=T, C=C, NPC=NPC, NBLK=NBLK, CC=CC, n_cores=n_cores,
        R=R, SL=SL, nch=nch, choff=choff, groups1=groups1, groups2=groups2,
    )
    return in_maps, meta


# ------------------------------------------------------------- device build


def build_nc(meta):
    N = meta["N"]
    T = meta["T"]
    C = meta["C"]
    NPC = meta["NPC"]
    NBLK = meta["NBLK"]
    CC = meta["CC"]
    KC = len(CC)
    n_cores = meta["n_cores"]
    R = meta["R"]
    SL = meta["SL"]
    nch = meta["nch"]
    choff = meta["choff"]
    groups1 = meta["groups1"]
    groups2 = meta["groups2"]
    rg = [list(range(n_cores))]
    sLast = N_SLICES - 1

    nc = bacc.Bacc(
        "TRN2", target_bir_lowering=False, debug=False, num_devices=n_cores,
        num_swdge_queues=4,
    )

    CK = CC[0][1]
    xsT = nc.dram_tensor("xsT", [CK, 3 * T, NPC], BF16, kind="ExternalInput")
    w1c = nc.dram_tensor("w1c", [KC, P, C], BF16, kind="ExternalInput")
    w2c = nc.dram_tensor("w2c", [KC, P, C], BF16, kind="ExternalInput")
    b1t = nc.dram_tensor("b1t", [P, C], F32, kind="ExternalInput")
    b2t = nc.dram_tensor("b2t", [P, C], F32, kind="ExternalInput")
    iot = nc.dram_tensor("iot", [P, P], BF16, kind="ExternalInput")
    dist = nc.dram_tensor("dist", [P, NBLK], F32, kind="ExternalInput")
    idx16 = [
        nc.dram_tensor(
            f"idx16_{s}", [P, int(choff[s, -1]) * 8], I16, kind="ExternalInput"
        )
        for s in range(N_SLICES)
    ]
    dlt = [
        nc.dram_tensor(
            f"dlt_{s}", [P, int(choff[s, -1])], BF16, kind="ExternalInput"
        )
        for s in range(N_SLICES)
    ]
    out_ext = nc.dram_tensor("out", [NPC, C], F32, kind="ExternalOutput")

    ACT = mybir.ActivationFunctionType
    slice_after = {_cdiv(R[s + 1], P) - 1: s for s in range(N_SLICES)}

    with tile.TileContext(nc) as tc:
        with (
            tc.tile_pool(name="dramp", bufs=1, space="DRAM") as dramp,
            tc.tile_pool(name="singles", bufs=1) as singles,
            tc.tile_pool(name="work", bufs=3) as wp,
            tc.tile_pool(name="xgp", bufs=2) as xgp,
            tc.tile_pool(name="msgs", bufs=4) as mp,
            tc.tile_pool(name="inds", bufs=4) as mi,
            tc.tile_pool(name="psA", bufs=2, space="PSUM") as psA,
            tc.tile_pool(name="psT", bufs=3, space="PSUM") as psT,
            tc.tile_pool(name="psB", bufs=2, space="PSUM") as psB,
        ):
            agin1 = [
                dramp.tile([SL[s], CE], BF16, name=f"agin1_{s}")
                for s in range(N_SLICES)
            ]
            hp1f = [
                dramp.tile(
                    [n_cores * SL[s], CE], BF16, addr_space="Shared",
                    name=f"hp1f_{s}",
                )
                for s in range(N_SLICES)
            ]
            agin2 = [
                dramp.tile([SL[s], CE], BF16, name=f"agin2_{s}")
                for s in range(N_SLICES)
            ]
            hp2f = [
                dramp.tile(
                    [n_cores * SL[s], CE], BF16, addr_space="Shared",
                    name=f"hp2f_{s}",
                )
                for s in range(N_SLICES)
            ]

            ident = singles.tile([P, P], BF16, name="ident")
            make_identity(nc, ident[:])
            w1sb = singles.tile([P, KC, C], BF16, name="w1sb")
            w2sb = singles.tile([P, KC, C], BF16, name="w2sb")
            for k in range(KC):
                nc.sync.dma_start(out=w1sb[:, k, :], in_=w1c[k])
                nc.sync.dma_start(out=w2sb[:, k, :], in_=w2c[k])
            b1sb = singles.tile([P, C], F32, name="b1sb")
            nc.sync.dma_start(out=b1sb[:], in_=b1t[:])
            b2sb = singles.tile([P, C], F32, name="b2sb")
            nc.sync.dma_start(out=b2sb[:], in_=b2t[:])
            iosb = singles.tile([P, P], BF16, name="iosb")
            nc.sync.dma_start(out=iosb[:], in_=iot[:])
            dissb = singles.tile([P, NBLK], F32, name="dissb")
            nc.sync.dma_start(out=dissb[:], in_=dist[:])
            idxsb = []
            dlsb = []
            for s in range(N_SLICES):
                t = singles.tile(
                    [P, int(choff[s, -1]) * 8], I16, name=f"idxsb{s}"
                )
                nc.sync.dma_start(out=t[:], in_=idx16[s][:])
                idxsb.append(t)
                t2 = singles.tile([P, int(choff[s, -1])], BF16, name=f"dlsb{s}")
                nc.sync.dma_start(out=t2[:], in_=dlt[s][:])
                dlsb.append(t2)

            hps1 = singles.tile([P, NBLK, C], BF16, name="hps1")
            hps2 = singles.tile([P, NBLK, C], BF16, name="hps2")
            hacc = singles.tile([P, NBLK, C], BF16, name="hacc")
            if NPC % P != 0:
                nc.vector.memset(hps1[:], 0.0)
                nc.vector.memset(hps2[:], 0.0)

            def ag_slice(agin, hpf, s):
                nc.gpsimd.collective_compute(
                    "AllGather",
                    mybir.AluOpType.bypass,
                    replica_groups=rg,
                    ins=[agin[s][:].opt()],
                    outs=[hpf[s][:].opt()],
                )

            def agin_write(agin, b, Pb, src):
                s = 0
                while R[s + 1] <= b * P:
                    s += 1
                o = b * P - R[s]
                nc.sync.dma_start(out=agin[s][o : o + Pb], in_=src)

            # ---------------- stage A: h = mean_t(x) @ W1.T + b1, prescale
            # x is host-transposed to [CK, 3T, NPC]; the t-sum runs on DVE
            # and the matmul consumes it directly as lhsT (no PE transposes).
            SGB = 2  # blocks per x load
            for g0 in range(0, NBLK, SGB):
                g1 = min(g0 + SGB, NBLK)
                W = min(SGB * P, NPC - g0 * P)
                xg = xgp.tile([CK, 3 * T, SGB * P], BF16, tag="xg")
                nc.sync.dma_start(
                    out=xg[:, :, :W], in_=xsT[:, :, g0 * P : g0 * P + W]
                )
                t01 = xgp.tile([CK, KC, SGB * P], BF16, tag="t01")
                t23 = xgp.tile([CK, KC, SGB * P], BF16, tag="t23")
                xsum = xgp.tile([CK, KC, SGB * P], BF16, tag="xsum")
                nc.vector.tensor_add(
                    out=t01[:, :, :W], in0=xg[:, 0:KC, :W], in1=xg[:, KC : 2 * KC, :W]
                )
                nc.vector.tensor_add(
                    out=t23[:, :, :W],
                    in0=xg[:, 2 * KC : 3 * KC, :W],
                    in1=xg[:, 3 * KC : 4 * KC, :W],
                )
                nc.vector.tensor_add(
                    out=xsum[:, :, :W], in0=t01[:, :, :W], in1=t23[:, :, :W]
                )
                for b in range(g0, g1):
                    Pb = min(P, NPC - b * P)
                    dcol = dissb[:Pb, b : b + 1]
                    o = (b - g0) * P
                    hpp = psA.tile([P, C], F32, tag="acc300")
                    for k in range(KC):
                        nc.tensor.matmul(
                            out=hpp[:Pb],
                            lhsT=xsum[:, k, o : o + Pb],
                            rhs=w1sb[:CK, k, :],
                            start=(k == 0),
                            stop=(k == KC - 1),
                        )
                    th = wp.tile([P, C], F32, tag="th")
                    nc.vector.tensor_add(out=th[:Pb], in0=hpp[:Pb], in1=b1sb[:Pb])
                    hp_t = wp.tile([P, CE], BF16, tag="hp")
                    nc.vector.memset(hp_t[:Pb, C:], 0.0)
                    nc.scalar.activation(
                        out=hp_t[:Pb, :C], in_=th[:Pb], func=ACT.Copy, scale=dcol
                    )
                    agin_write(agin1, b, Pb, hp_t[:Pb])
                    nc.scalar.activation(
                        out=hps1[:Pb, b, :],
                        in_=hp_t[:Pb, :C],
                        func=ACT.Copy,
                        scale=dcol,
                    )
                    if b in slice_after:
                        ag_slice(agin1, hp1f, slice_after[b])

            # ------------- gather slice s chunks for blocks [g0, g1)
            qn = [0]

            def gather(hpf, s, g0, g1):
                c0, c1 = int(choff[s, g0]), int(choff[s, g1])
                ncols = c1 - c0
                msg = mp.tile([P, MAXCH, CE], BF16, tag="msg")
                nidx = ncols * P
                nc.gpsimd.dma_gather(
                    out_ap=msg[:, :ncols, :],
                    in_ap=hpf[s][:],
                    idxs_ap=idxsb[s][:, c0 * 8 : c1 * 8],
                    num_idxs=nidx,
                    num_idxs_reg=nidx,
                    elem_size=CE,
                    queue_num=qn[0],
                )
                qn[0] = (qn[0] + 1) % 4
                return msg

            # all indicators for blocks [g0, g1) of slice s in one DVE op
            def indicators(s, g0, g1):
                c0, c1 = int(choff[s, g0]), int(choff[s, g1])
                ncols = c1 - c0
                indall = mp.tile([P, MAXCH, P], BF16, tag="indall")
                nc.vector.tensor_tensor(
                    out=indall[:, :ncols, :],
                    in0=iosb[:].unsqueeze(1).to_broadcast([P, ncols, P]),
                    in1=dlsb[s][:, c0:c1].unsqueeze(2).to_broadcast(
                        [P, ncols, P]
                    ),
                    op=mybir.AluOpType.is_equal,
                )
                return indall

            def prop_pass1(hpf, hps):
                for g0, g1 in groups1:
                    msgs = [gather(hpf, s, g0, g1) for s in range(N_SLICES - 1)]
                    inds = [
                        indicators(s, g0, g1) for s in range(N_SLICES - 1)
                    ]
                    for b in range(g0, g1):
                        pp = psB.tile([P, C], F32, tag="pp")
                        first = True
                        for s in range(N_SLICES - 1):
                            mcol0 = int(choff[s, b]) - int(choff[s, g0])
                            for ch in range(int(nch[s, b])):
                                last = (
                                    s == N_SLICES - 2
                                    and ch == int(nch[s, b]) - 1
                                )
                                nc.tensor.matmul(
                                    out=pp[:],
                                    lhsT=inds[s][:, mcol0 + ch, :],
                                    rhs=msgs[s][:, mcol0 + ch, :C],
                                    start=first,
                                    stop=last,
                                )
                                first = False
                        nc.vector.scalar_tensor_tensor(
                            out=hacc[:, b, :],
                            in0=pp[:],
                            scalar=dissb[:, b : b + 1],
                            in1=hps[:, b, :],
                            op0=mybir.AluOpType.mult,
                            op1=mybir.AluOpType.add,
                        )

            def prop_pass2_block(msg, indall, g0, b):
                pp = psB.tile([P, C], F32, tag="pp")
                mcol0 = int(choff[sLast, b]) - int(choff[sLast, g0])
                nb = int(nch[sLast, b])
                for ch in range(nb):
                    nc.tensor.matmul(
                        out=pp[:],
                        lhsT=indall[:, mcol0 + ch, :],
                        rhs=msg[:, mcol0 + ch, :C],
                        start=(ch == 0),
                        stop=(ch == nb - 1),
                    )
                t1 = wp.tile([P, C], F32, tag="t1")
                nc.vector.scalar_tensor_tensor(
                    out=t1[:],
                    in0=pp[:],
                    scalar=dissb[:, b : b + 1],
                    in1=hacc[:, b, :],
                    op0=mybir.AluOpType.mult,
                    op1=mybir.AluOpType.add,
                )
                return t1

            # ---------------- layer 1 prop + layer 2 linear
            prop_pass1(hp1f, hps1)
            for g0, g1 in groups2:
                msg = gather(hp1f, sLast, g0, g1)
                indall = indicators(sLast, g0, g1)
                for b in range(g0, g1):
                    Pb = min(P, NPC - b * P)
                    t1 = prop_pass2_block(msg, indall, g0, b)
                    h1 = wp.tile([P, C], BF16, tag="h1")
                    nc.vector.scalar_tensor_tensor(
                        out=h1[:],
                        in0=t1[:],
                        scalar=0.01,
                        in1=t1[:],
                        op0=mybir.AluOpType.mult,
                        op1=mybir.AluOpType.max,
                    )
                    h2p = psA.tile([P, C], F32, tag="acc300")
                    hTs = []
                    for k, (c0, cs) in enumerate(CC):
                        ptr2 = psT.tile([P, P], BF16, tag="ptr")
                        nc.tensor.transpose(
                            out=ptr2[:cs, :],
                            in_=h1[:, c0 : c0 + cs],
                            identity=ident[:],
                        )
                        hT = wp.tile([P, P], BF16, tag=f"hT{k}")
                        nc.scalar.copy(out=hT[:cs, :], in_=ptr2[:cs, :])
                        hTs.append(hT)
                    for k, (c0, cs) in enumerate(CC):
                        nc.tensor.matmul(
                            out=h2p[:],
                            lhsT=hTs[k][:cs, :],
                            rhs=w2sb[:cs, k, :],
                            start=(k == 0),
                            stop=(k == KC - 1),
                        )
                    t2 = wp.tile([P, C], F32, tag="t2")
                    nc.vector.tensor_add(out=t2[:], in0=h2p[:], in1=b2sb[:])
                    hp2_t = wp.tile([P, CE], BF16, tag="hp2")
                    nc.vector.memset(hp2_t[:Pb, C:], 0.0)
                    nc.scalar.activation(
                        out=hp2_t[:Pb, :C],
                        in_=t2[:Pb],
                        func=ACT.Copy,
                        scale=dissb[:Pb, b : b + 1],
                    )
                    agin_write(agin2, b, Pb, hp2_t[:Pb])
                    nc.scalar.activation(
                        out=hps2[:Pb, b, :],
                        in_=hp2_t[:Pb, :C],
                        func=ACT.Copy,
                        scale=dissb[:Pb, b : b + 1],
                    )
                    if b in slice_after:
                        ag_slice(agin2, hp2f, slice_after[b])

            # ---------------- layer 2 prop -> output
            prop_pass1(hp2f, hps2)
            for g0, g1 in groups2:
                msg = gather(hp2f, sLast, g0, g1)
                indall = indicators(sLast, g0, g1)
                for b in range(g0, g1):
                    Pb = min(P, NPC - b * P)
                    ot = prop_pass2_block(msg, indall, g0, b)
                    nc.sync.dma_start(out=out_ext[b * P : b * P + Pb], in_=ot[:Pb])

    nc.compile()
    return nc


# ------------------------------------------------------------------ runner

_CACHE = {}


def run(x, edge_index, W1, b1, W2, b2, n_cores=N_CORES, trace=False):
    in_maps, meta = prep_inputs(x, edge_index, W1, b1, W2, b2, n_cores)
    key = (
        meta["N"], meta["T"], meta["C"], n_cores,
        tuple(map(tuple, meta["nch"])),
    )
    if key not in _CACHE:
        _CACHE[key] = build_nc(meta)
    nc = _CACHE[key]
    res = bass_utils.run_bass_kernel_spmd(
        nc, in_maps, core_ids=list(range(n_cores)), trace=trace
    )
    outs = [np.asarray(res.results[c]["out"]) for c in range(n_cores)]
    full = np.concatenate(outs, axis=0).astype(np.float32)
    return full, res


def kernel(x, edge_index, W1, b1, W2, b2):
    x = np.asarray(x)
    edge_index = np.asarray(edge_index)
    full, _ = run(
        np.asarray(x, np.float32),
        edge_index,
        np.asarray(W1, np.float32),
        np.asarray(b1, np.float32),
        np.asarray(W2, np.float32),
        np.asarray(b2, np.float32),
    )
    return full
